# revision 75
# baseline (speedup 1.0000x reference)
"""Trainium2 Bass kernel for nn_CNNConcatLinear (B=1024, N=24, PD=2, C=512).

Strategy: pure data-parallel over batch (128 per core x 8 cores).

Per core, channels-on-partitions layouts with fp8e4m3 DoubleRow matmuls
(2x128 contraction per instruction at 0.5 cycles/output-row) for every
heavy contraction: the c1 ConcatSquashLinear, the 6 merged convolutions,
c3, c4, and the phase-B gate/hyper-bias matmuls. The final cl layer stays
in f32r (its hyper path dominates the error budget), with bf16 gate
matmuls for cl.

  phase A: new_ctx = context + (sum_j e_j t_j)/(sum_j e_j) -- the two 1x1
           convs + softmax collapse algebraically into a 3x3 system folded
           on host.
  phase B: all gates/hyper-biases as [feature, batch] fp8-DR matmuls from
           new_ctx^T; batched group epilogues (sigmoid on Act, copies on
           DVE) when the gate biases are all zero (they are, per the
           reference init).
  phase C: pipeline over batch chunks of 16:
           c1 (one DR matmul incl. the bias via a ones-row) -> gated X
           [f, b, n] fp8 -> convs as 28 (co, tap) blocks x 4 ci-pair DR
           matmuls -> c3 (4 m-chunks x 4 pair DR + 1 bias-fold DR against
           a delta tile; conv bias + positional encoding folded into the
           c3 bias table on host) -> c4 -> cl (f32r).

Output is written DMA-friendly as [PD, BLOC*N] per core and transposed
back to [BLOC, N, PD] on host.
"""

import math
import os

import numpy as np
import ml_dtypes

B, N, PD, C = 1024, 24, 2, 512
F = 2 * C
NCORES = 8
BLOC = B // NCORES          # 128 batch per core
BC = 16                     # batch chunk
NBC = BLOC // BC            # 8 chunks
PADL = 8
NW = N + 2 * PADL           # 40 (padded X width)
FREE = BC * N               # 384

TAPS = {
    0: [0], 1: [0], 2: [0], 3: [0],
    4: [0, -1, 1], 5: [0, -1, 1],
    6: [0, -1, 1, -2, 2, -3, 3],
    7: [0, -1, 1, -2, 2, -3, 3, -4, 4, -5, 5],
}
BLK = {}
for _co in range(8):
    for _d in TAPS[_co]:
        BLK[(_co, _d)] = len(BLK)
NBLK = len(BLK)             # 28

NP8 = ml_dtypes.float8_e4m3
NPBF = ml_dtypes.bfloat16

LAST_RESULTS = None         # BassKernelResults from the most recent run


def _pe_table():
    pos = np.arange(N, dtype=np.float32)[:, None]
    div = np.exp(np.arange(0, F, 2, dtype=np.float32) * (-np.log(10000.0) / F))
    pe = np.zeros((N, F), dtype=np.float32)
    pe[:, 0::2] = np.sin(pos * div)
    pe[:, 1::2] = np.cos(pos * div)
    return pe


def _f32(a):
    return np.ascontiguousarray(np.asarray(a, dtype=np.float32))


def _f8(a):
    return np.ascontiguousarray(np.asarray(np.asarray(a, np.float32), dtype=NP8))


def _build(host, num_devices=NCORES):
    import concourse.bass as bass
    import concourse.mybir as mybir
    import concourse.tile as tile
    from concourse import bacc
    from concourse.masks import make_identity

    f32 = mybir.dt.float32
    f32r = mybir.dt.float32r
    f8 = mybir.dt.float8e4
    bf16 = mybir.dt.bfloat16
    DR = mybir.MatmulPerfMode.DoubleRow
    AluOp = mybir.AluOpType
    Act = mybir.ActivationFunctionType

    M3, v3, s3 = host["M3"], host["v3"], host["s3"]
    GB_ZERO = host["gb_zero"]

    nc = bacc.Bacc("TRN2", target_bir_lowering=False, debug=False,
                   num_devices=num_devices)

    def din(name, shape, dt):
        return nc.dram_tensor(name, list(shape), dt, kind="ExternalInput").ap()

    ctx_d = din("ctx", [BLOC, C], f32)
    smalls_d = din("smalls", [BLOC, 46], f32)
    xt8_d = din("xt8", [2, 2, F + BLOC * N], f8)
    wg8_d = din("wg8", [128, 28, 2, 2, 128], f8)
    wgl_d = din("wgl", [128, 4, 2, 2], bf16)
    convt8_d = din("convt8", [128, 4, NBLK, 2, 128], f8)
    c3w8_d = din("c3w8", [128, 4, 5, 2, 128], f8)
    c4w8_d = din("c4w8", [128, 2, 2, 2, 128], f8)
    clwt_d = din("clwt", [128, 2, PD], f32r)
    d8_d = din("d8", [128, NBC, 2, FREE], f8)
    out_d = nc.dram_tensor("out", [PD, BLOC * N], f32,
                           kind="ExternalOutput").ap()
    DEBUG = bool(int(os.environ.get("KERNEL_DEBUG", "0")))
    if DEBUG:
        dbg_nctx = nc.dram_tensor("dbg_nctx", [128, C], f32,
                                  kind="ExternalOutput").ap()
        dbg_g = nc.dram_tensor("dbg_g", [128, 2048], f32,
                               kind="ExternalOutput").ap()
        dbg_x = nc.dram_tensor("dbg_x", [4, 128, 2, BC, NW], f32,
                               kind="ExternalOutput").ap()
        dbg_y = nc.dram_tensor("dbg_y", [4, 128, 2, BC, N], f32,
                               kind="ExternalOutput").ap()
        dbg_t3 = nc.dram_tensor("dbg_t3", [2, 128, 2, BC, N], f32,
                                kind="ExternalOutput").ap()
        dbg_t4 = nc.dram_tensor("dbg_t4", [2, 128, BC, N], f32,
                                kind="ExternalOutput").ap()
        dbg_gw = nc.dram_tensor("dbg_gw", [4, 128, 2, 2, 128], f32,
                                kind="ExternalOutput").ap()
        dbg_n8 = nc.dram_tensor("dbg_n8", [128, 4, 128], f32,
                                kind="ExternalOutput").ap()
        dbg_wgd = nc.dram_tensor("dbg_wgd", [128, 4, 2, 2, 128], f8,
                                 kind="ExternalOutput").ap()

    with tile.TileContext(nc) as tc:
        import contextlib
        est = contextlib.ExitStack()
        with est:
            wp = est.enter_context(tc.tile_pool(name="wp", bufs=1))
            gout = est.enter_context(tc.tile_pool(name="gout", bufs=1))

            # ---------- persistent small tiles + their DMAs ----------
            smalls = wp.tile([128, 46], f32, tag="smalls")
            nc.sync.dma_start(smalls[:], smalls_d[:])
            beta_t0 = smalls[:, 0:1]
            gbias_s = smalls[:, 1:30]
            c4b_s = smalls[:, 30:32]
            clb_s = smalls[0:PD, 32:33]
            ctx_t0 = wp.tile([128, C], f32, tag="ctx")
            nc.sync.dma_start(ctx_t0[:], ctx_d[:])
            ident = wp.tile([128, 128], f32, tag="ident")
            make_identity(nc, ident[:])

            # gate/hyper output tiles [feature_part, chunk*128 + b]
            g1b1_s = gout.tile([128, 2048], f32, tag="g1b1")
            g3h3_s = gout.tile([128, 1024], f32, tag="g3h3")
            g4h4_s = gout.tile([128, 512], f32, tag="g4h4")
            gl_s = gout.tile([PD, 128], f32, tag="gl")
            hl_s = gout.tile([PD, 128], f32, tag="hl")

            # ---------- conv weight tiles (fp8 pairs) ----------
            convw_all = wp.tile([128, 4, NBLK, 2, 128], f8, tag="convw")
            convw_s = [convw_all[:, P] for P in range(4)]

            # ---------- phase-C SBUF pools (open early so c1(0) can run in B)
            xp = est.enter_context(tc.tile_pool(name="xp", bufs=1))
            tmpp = est.enter_context(tc.tile_pool(name="tmpp", bufs=6))
            dbgp = est.enter_context(tc.tile_pool(name="dbgp", bufs=1)) \
                if DEBUG else None

            def bcast(ap_2d, np_=N):
                return ap_2d.unsqueeze(2).broadcast_to(
                    [ap_2d.shape[0], BC, np_])


            def _c1_step(bc, fc, pspool, pstag="c1"):
                cs = bc * BC
                X_t = X_gen[bc % 2]
                ps1 = pspool.tile([128, BC, N], f32, tag=pstag)
                nc.tensor.matmul(
                    ps1[:].rearrange("p b n -> p (b n)"), xt8_w[:, :, fc, :],
                    xt8_s[:, :, F + cs * N:F + cs * N + FREE],
                    start=True, stop=True, perf_mode=DR)
                g1 = bcast(g1b1_s[:, fc * 128 + cs:fc * 128 + cs + BC])
                b1 = bcast(g1b1_s[:, 1024 + fc * 128 + cs:1024 + fc * 128 + cs + BC])
                tmp = tmpp.tile([128, BC, N], f32, tag="c1tmp")
                nc.vector.tensor_mul(tmp[:], ps1[:], g1)
                xi = X_t[fc // 2][:, fc % 2, :, PADL:PADL + N]
                (nc.gpsimd if fc % 2 == 0 else nc.vector).tensor_add(
                    xi, tmp[:], b1)

            # ---------- phase A: new_ctx ----------
            with tc.tile_pool(name="pa", bufs=1) as pap:
                ctx_t = ctx_t0
                beta_t = beta_t0

                sinb = pap.tile([128, 1], f32, tag="sinb")
                nc.scalar.activation(sinb[:], beta_t[:], Act.Sin)
                cosb = pap.tile([128, 1], f32, tag="cosb")
                nc.scalar.activation(cosb[:], beta_t[:], Act.Sin,
                                     bias=smalls[:, 45:46])

                u = pap.tile([128, 3], f32, tag="u")
                cm3_0, cm3_1, cm3_2 = (smalls[:, 33 + 3 * k:36 + 3 * k]
                                       for k in range(3))
                v3_t = smalls[:, 42:45]
                nc.vector.scalar_tensor_tensor(u[:], cm3_0, beta_t[:], v3_t,
                                               AluOp.mult, AluOp.add)
                nc.vector.scalar_tensor_tensor(u[:], cm3_1, sinb[:], u[:],
                                               AluOp.mult, AluOp.add)
                nc.vector.scalar_tensor_tensor(u[:], cm3_2, cosb[:], u[:],
                                               AluOp.mult, AluOp.add)

                psa_cm = tc.tile_pool(name="ps_a", bufs=4, space="PSUM")
                psa = psa_cm.__enter__()
                ej = psa.tile([128, C], f32, tag="ej")
                z = pap.tile([128, C], f32, tag="z")
                num = pap.tile([128, C], f32, tag="num")
                tvec = [beta_t, sinb, cosb]
                for j in range(3):
                    nc.scalar.activation(ej[:], ctx_t[:], Act.Exp,
                                         bias=u[:, j:j + 1], scale=float(s3[j]))
                    if j == 0:
                        nc.vector.tensor_copy(z[:], ej[:])
                        nc.vector.tensor_scalar(num[:], ej[:], tvec[j][:], None,
                                                AluOp.mult)
                    else:
                        nc.vector.tensor_add(z[:], z[:], ej[:])
                        nc.vector.scalar_tensor_tensor(num[:], ej[:], tvec[j][:],
                                                       num[:], AluOp.mult,
                                                       AluOp.add)
                nc.vector.reciprocal(z[:], z[:])
                nc.vector.tensor_mul(num[:], num[:], z[:])
                nctx = ctx_t
                nc.vector.tensor_add(nctx[:], ctx_t[:], num[:])

                nctx8 = wp.tile([128, 4, 128], f8, tag="nctx8")
                nctxbf = wp.tile([128, 4, 128], bf16, tag="nctxbf")
                for kb in range(4):
                    pst = psa.tile([128, 128], f32, tag="tr")
                    nc.tensor.transpose(pst[:],
                                        nctx[:, kb * 128:(kb + 1) * 128],
                                        ident[:])
                    nc.vector.tensor_copy(nctx8[:, kb, :], pst[:])
                    nc.scalar.copy(nctxbf[:, kb, :], pst[:])
                if DEBUG:
                    dbg_nx0 = nctx
                psa_cm.__exit__(None, None, None)
            nctx8p = nctx8[:].rearrange("p (P i) b -> p P i b", P=2)

            # ---------- bulk weight/data DMAs: one ordered SP stream
            # (DMA transfers serialize on the shared DMA engines; order by
            # first use). xt8 rides the Pool SWDGE queue in parallel.
            xt8_s = wp.tile([2, 2, F + BLOC * N], f8, tag="xt8")
            nc.gpsimd.dma_start(xt8_s[:], xt8_d[:])
            gw_all = wp.tile([128, 28, 2, 2, 128], f8, tag="gw")
            for q in range(4):
                nc.sync.dma_start(gw_all[:, 7 * q:7 * q + 7],
                                  wg8_d[:, 7 * q:7 * q + 7])
            wgl_s = wp.tile([128, 4, 2, 2], bf16, tag="wgl")
            nc.sync.dma_start(wgl_s[:], wgl_d[:])
            for q in range(4):
                nc.sync.dma_start(convw_all[:, :, 7 * q:7 * q + 7],
                                  convt8_d[:, :, 7 * q:7 * q + 7])
            c3w_all = wp.tile([128, 4, 5, 2, 128], f8, tag="c3w")
            nc.sync.dma_start(c3w_all[:], c3w8_d[:])
            c4w_all = wp.tile([128, 2, 2, 2, 128], f8, tag="c4w")
            nc.sync.dma_start(c4w_all[:], c4w8_d[:])
            d8_s = wp.tile([128, NBC, 2, FREE], f8, tag="d8")
            nc.sync.dma_start(d8_s[:], d8_d[:])
            clw_all = wp.tile([128, 2, PD], f32r, tag="clw")
            nc.sync.dma_start(clw_all[:], clwt_d[:])

            X_gen = []
            for gen in range(2):
                gtiles = []
                for P in range(4):
                    Xf = xp.tile([128, 2, BC, NW], f8, tag=f"x{gen}_{P}")
                    nc.gpsimd.memset(Xf[:, :, :, 0:PADL].bitcast(f32), 0.0)
                    nc.gpsimd.memset(Xf[:, :, :, PADL + N:NW].bitcast(f32), 0.0)
                    gtiles.append(Xf)
                X_gen.append(gtiles)
            xt8_w = xt8_s[:, :, 0:F].rearrange("p i (fc m) -> p i fc m", fc=8)

            # ---------- phase B: gates ----------
            ps_c1 = est.enter_context(
                tc.tile_pool(name="ps_c1", bufs=2, space="PSUM"))
            with tc.tile_pool(name="ps_b", bufs=5, space="PSUM") as pbp:
                # 4 chunk-serial chains per bank tile, one batched drain per
                # group (safe: chains are serial and fully stopped before the
                # group read)
                GDEST = {}
                for c in range(28):
                    if c < 8:
                        GDEST[c] = (g1b1_s, c * 128, True)
                    elif c < 16:
                        GDEST[c] = (g1b1_s, 1024 + (c - 8) * 128, False)
                    elif c < 20:
                        GDEST[c] = (g3h3_s, (c - 16) * 128, True)
                    elif c < 24:
                        GDEST[c] = (g3h3_s, 512 + (c - 20) * 128, False)
                    elif c < 26:
                        GDEST[c] = (g4h4_s, (c - 24) * 128, True)
                    else:
                        GDEST[c] = (g4h4_s, 256 + (c - 26) * 128, False)
                gp_bank = None
                ORDER = [8, 9, 10, 11, 0, 1, 2, 3, 12, 13, 14, 15, 4, 5, 6, 7]
                ORDER += list(range(16, 28))
                C1_B = {3: [0, 1, 2, 3], 7: [4, 5, 6, 7]}
                for idx, c in enumerate(ORDER):
                    if idx % 4 == 0:
                        gp_bank = pbp.tile([128, 4, 128], f32, tag="gps")
                    for P in range(2):
                        nc.tensor.matmul(gp_bank[:, idx % 4, :],
                                         gw_all[:, c, P], nctx8p[:, P],
                                         start=(P == 0), stop=(P == 1),
                                         perf_mode=DR)
                    if idx % 4 == 3 and GB_ZERO and c != 27:
                        dst_t, off, is_g = GDEST[c - 3]
                        dst = dst_t[:, off:off + 512]
                        src = gp_bank[:].rearrange("p c b -> p (c b)")
                        if is_g:
                            nc.scalar.activation(dst, src, Act.Sigmoid)
                        else:
                            nc.vector.tensor_copy(dst, src)
                    elif idx % 4 == 3 and GB_ZERO:
                        nc.scalar.activation(
                            g4h4_s[:, 0:256],
                            gp_bank[:, 0:2].rearrange("p c b -> p (c b)"),
                            Act.Sigmoid)
                        nc.vector.tensor_copy(
                            g4h4_s[:, 256:512],
                            gp_bank[:, 2:4].rearrange("p c b -> p (c b)"))
                    elif idx % 4 == 3:
                        for cc in range(c - 3, c + 1):
                            dst_t, off, is_g = GDEST[cc]
                            dst = dst_t[:, off:off + 128]
                            if is_g:
                                nc.scalar.activation(
                                    dst, gp_bank[:, cc % 4, :], Act.Sigmoid,
                                    bias=gbias_s[:, cc:cc + 1])
                            else:
                                nc.vector.tensor_copy(dst, gp_bank[:, cc % 4, :])
                    for fc in C1_B.get(c if idx % 4 == 3 else -1, []):
                        _c1_step(0, fc, ps_c1)

                # cl gates in bf16: two [2, 128] psums (partition-0 aligned)
                psl_g_full = pbp.tile([128, 128], f32, tag="gps")
                psl_h_full = pbp.tile([128, 128], f32, tag="gps")
                psl_g = psl_g_full[0:2]
                psl_h = psl_h_full[0:2]
                for kb in range(4):
                    nc.tensor.matmul(psl_g[:], wgl_s[:, kb, 0, :],
                                     nctxbf[:, kb, :],
                                     start=(kb == 0), stop=(kb == 3))
                for kb in range(4):
                    nc.tensor.matmul(psl_h[:], wgl_s[:, kb, 1, :],
                                     nctxbf[:, kb, :],
                                     start=(kb == 0), stop=(kb == 3))
                nc.scalar.activation(gl_s[:], psl_g[:], Act.Sigmoid,
                                     bias=gbias_s[0:2, 28:29])
                nc.scalar.copy(hl_s[:], psl_h[:])

                # h/g folds: a*g + h == (a + h/g)*g, so h/g (and c4b) ride the
                # psum via delta-tile matmuls and each CSL epilogue is one mul
                h3g = wp.tile([128, 512], f32, tag="h3g")
                nc.vector.reciprocal(h3g[:], g3h3_s[:, 0:512])
                nc.vector.tensor_mul(h3g[:], h3g[:], g3h3_s[:, 512:1024])
                h4g = wp.tile([128, 256], f32, tag="h4g")
                nc.vector.reciprocal(h4g[:], g4h4_s[:, 0:256])
                nc.vector.tensor_mul(h4g[:], h4g[:], g4h4_s[:, 256:512])
                for mc in range(2):
                    nc.vector.tensor_scalar(
                        h4g[:, mc * 128:(mc + 1) * 128],
                        h4g[:, mc * 128:(mc + 1) * 128],
                        c4b_s[:, mc:mc + 1], None, AluOp.add)
                # write folds into plane 1 of the combined delta-weight
                # tiles (plane 0 carries the c3 bias / zeros)
                c4f = wp.tile([128, 2, 2, 128], f8, tag="c4f")
                nc.vector.memset(c4f[:, :, 0, :].bitcast(f32), 0.0)
                trh_cm = tc.tile_pool(name="ps_t", bufs=1, space="PSUM")
                trhp = trh_cm.__enter__()
                for m in range(4):
                    pst = trhp.tile([128, 128], f32, tag="trh")
                    nc.tensor.transpose(pst[:], h3g[:, m * 128:(m + 1) * 128],
                                        ident[:])
                    nc.vector.tensor_copy(c3w_all[:, m, 4, 1, :], pst[:])
                for mc in range(2):
                    pst = trhp.tile([128, 128], f32, tag="trh")
                    nc.tensor.transpose(pst[:], h4g[:, mc * 128:(mc + 1) * 128],
                                        ident[:])
                    nc.vector.tensor_copy(c4f[:, mc, 1, :], pst[:])
                trh_cm.__exit__(None, None, None)
            if DEBUG:
                nc.sync.dma_start(dbg_nctx[:], dbg_nx0[:])
                nc.sync.dma_start(dbg_g[:], g1b1_s[:])
                for c in range(4):
                    stg_gw = dbgp.tile([128, 2, 2, 128], f32, tag="dgw",
                                       name="stggw")
                    nc.scalar.copy(stg_gw[:], gw_all[:, c])
                    nc.sync.dma_start(dbg_gw[c], stg_gw[:])
                stg_n8 = dbgp.tile([128, 4, 128], f32, tag="dn8", name="stgn8")
                nc.scalar.copy(stg_n8[:], nctx8[:])
                nc.sync.dma_start(dbg_n8[:], stg_n8[:])
                nc.sync.dma_start(dbg_wgd[:], wg8_d[:, 0:4])

            # ---------- phase C ----------
            C1_AT = {1: [0], 2: [1], 3: [2, 3], 4: [4], 5: [5], 6: [6], 7: [7]}
            with (
                tc.tile_pool(name="yp", bufs=2) as yp,
                tc.tile_pool(name="t3p", bufs=2) as t3p,
                tc.tile_pool(name="t4p", bufs=2) as t4p,
                tc.tile_pool(name="ofp", bufs=3) as ofp,
                tc.tile_pool(name="ps_cv", bufs=4, space="PSUM") as ps_cv,
                tc.tile_pool(name="ps_c3", bufs=2, space="PSUM") as ps_c3,
            ):
                def _dbg_dump(dst, tiles, shape):
                    for i, t in enumerate(tiles):
                        stg = dbgp.tile(shape, f32, tag="dbg", name="dbgstg")
                        nc.scalar.copy(stg[:], t[:])
                        nc.sync.dma_start(dst[i], stg[:])

                def emit_c3(bc, Y_t):
                    cs = bc * BC
                    T3_t = [t3p.tile([128, 2, BC, N], f8, tag=f"t3{P}",
                                     name=f"t3{P}") for P in range(2)]
                    for m in range(4):
                        ps3 = ps_c3.tile([128, BC, N], f32, tag="c3")
                        ps3f = ps3[:].rearrange("p b n -> p (b n)")
                        for P in range(4):
                            nc.tensor.matmul(
                                ps3f, c3w_all[:, m, P],
                                Y_t[P][:].rearrange("p i b n -> p i (b n)"),
                                start=(P == 0), stop=False, perf_mode=DR)
                        nc.tensor.matmul(
                            ps3f, c3w_all[:, m, 4], d8_s[:, bc],
                            start=False, stop=True, perf_mode=DR)
                        g3 = bcast(g3h3_s[:, m * 128 + cs:m * 128 + cs + BC])
                        nc.vector.tensor_mul(T3_t[m // 2][:, m % 2], ps3[:], g3)
                    if DEBUG and bc == 0:
                        _dbg_dump(dbg_t3, T3_t, [128, 2, BC, N])
                    return T3_t

                def emit_c4(bc, T3_t):
                    cs = bc * BC
                    T4_t = [t4p.tile([128, BC, N], f32r, tag=f"t4{mc}",
                                     name=f"t4{mc}") for mc in range(2)]
                    for mc in range(2):
                        ps4 = ps_c3.tile([128, BC, N], f32, tag="c3")
                        ps4f = ps4[:].rearrange("p b n -> p (b n)")
                        for P in range(2):
                            nc.tensor.matmul(
                                ps4f, c4w_all[:, mc, P],
                                T3_t[P][:].rearrange("p i b n -> p i (b n)"),
                                start=(P == 0), stop=False, perf_mode=DR)
                        nc.tensor.matmul(
                            ps4f, c4f[:, mc], d8_s[:, bc],
                            start=False, stop=True, perf_mode=DR)
                        g4 = bcast(g4h4_s[:, mc * 128 + cs:mc * 128 + cs + BC])
                        nc.vector.tensor_mul(T4_t[mc][:], ps4[:], g4)
                    if DEBUG and bc == 0:
                        _dbg_dump(dbg_t4, T4_t, [128, BC, N])
                    return T4_t

                def emit_cl(bc, T4_t):
                    cs = bc * BC
                    psl_full = ps_c3.tile([128, BC, N], f32, tag="c3")
                    psl = psl_full[0:PD]
                    for k in range(2):
                        nc.tensor.matmul(psl[:], clw_all[:, k, :], T4_t[k][:],
                                         start=(k == 0), stop=(k == 1))
                    OF = ofp.tile([PD, BC, N], f32, tag="of")
                    gl = gl_s[:, cs:cs + BC].unsqueeze(2).broadcast_to([PD, BC, N])
                    hl = hl_s[:, cs:cs + BC].unsqueeze(2).broadcast_to([PD, BC, N])
                    nc.vector.scalar_tensor_tensor(OF[:], psl[:], clb_s[:], gl,
                                                   AluOp.add, AluOp.mult)
                    (nc.vector if bc >= NBC - 2 else nc.gpsimd).tensor_add(
                        OF[:], OF[:], hl)
                    nc.sync.dma_start(
                        out_d[:, bc * FREE:(bc + 1) * FREE],
                        OF[:].rearrange("p b n -> p (b n)"))

                # 1-chunk skew: chunk bc-1's c3/c4/cl matmuls are emitted
                # between chunk bc's conv chains so the PE never waits on a
                # freshly-written epilogue output
                stage = {}
                for bc in range(NBC):
                    X_t = X_gen[bc % 2]
                    if DEBUG and bc == 0:
                        _dbg_dump(dbg_x, X_t, [128, 2, BC, NW])

                    Y_t = [yp.tile([128, 2, BC, N], f8, tag=f"y{P}",
                                   name=f"y{P}") for P in range(4)]
                    for co in range(8):
                        if bc + 1 < NBC:
                            for fc in C1_AT.get(co, []):
                                _c1_step(bc + 1, fc, ps_c1)
                        if co == 0 and bc >= 2:
                            emit_cl(bc - 2, stage[bc - 2]["T4"])
                        if bc >= 1:
                            prev = stage[bc - 1]
                            if co == 1:
                                prev["T3"] = emit_c3(bc - 1, prev["Y"])
                            elif co == 6:
                                prev["T4"] = emit_c4(bc - 1, prev["T3"])
                        psc = ps_cv.tile([128, BC, N], f32, tag="conv")
                        mms = [(d, P) for d in TAPS[co] for P in range(4)]
                        for i, (d, P) in enumerate(mms):
                            nc.tensor.matmul(
                                psc[:], convw_s[P][:, BLK[(co, d)], :, :],
                                X_t[P][:, :, :, PADL + d:PADL + d + N],
                                start=(i == 0), stop=(i == len(mms) - 1),
                                perf_mode=DR)
                        nc.scalar.copy(Y_t[co // 2][:, co % 2], psc[:])
                    stage[bc] = {"Y": Y_t}
                    if DEBUG and bc == 0:
                        _dbg_dump(dbg_y, Y_t, [128, 2, BC, N])

                emit_cl(NBC - 2, stage[NBC - 2]["T4"])
                last = stage[NBC - 1]
                last["T3"] = emit_c3(NBC - 1, last["Y"])
                last["T4"] = emit_c4(NBC - 1, last["T3"])
                emit_cl(NBC - 1, last["T4"])

    nc.compile()
    return nc


def _build_and_run(host, in_maps, trace):
    from concourse.bass_utils import run_bass_kernel_spmd

    nc = _build(host)
    res = run_bass_kernel_spmd(
        nc, in_maps, core_ids=list(range(NCORES)), trace=trace,
        trace_cores=list(range(NCORES)) if trace else None,
        stitch_traces=bool(trace and NCORES > 1))
    return res


def _host_prep(**inputs):
    x = _f32(inputs["x"])
    beta = _f32(inputs["beta"])
    context = _f32(inputs["context"])
    g = {k: np.asarray(v, dtype=np.float64) for k, v in inputs.items()
         if k not in ("x", "beta", "context")}

    # --- algebraic folds (host, tiny) ---
    embW = g["emb_w"][:, :, 0]            # [64, 3]
    dembW = g["demb_w"][:, :, 0]          # [3, 64]
    M3 = dembW @ embW                     # [3, 3]
    v3 = dembW @ g["emb_b"] + g["demb_b"]
    s3 = M3.sum(axis=1)

    pe = _pe_table().astype(np.float64)   # [N, F]

    # gate weight matrix WG [C, 28*128] then DR pair layout
    WG = np.concatenate([
        g["c1_gw"].T, g["c1_hw"].T, g["c3_gw"].T, g["c3_hw"].T,
        g["c4_gw"].T, g["c4_hw"].T], axis=1).astype(np.float32)  # [512, 3584]
    wg8 = _f8(WG.reshape(2, 2, 128, 28, 128).transpose(2, 3, 0, 1, 4))
    WGL = np.stack([g["cl_gw"].T, g["cl_hw"].T], axis=1)  # [512, 2, 2]
    wgl = np.ascontiguousarray(
        WGL.reshape(4, 128, 2, 2).transpose(1, 0, 2, 3).astype(NPBF))

    gbias = np.zeros(29 * 128, np.float32)
    gbias[0:1024] = g["c1_gb"]
    gbias[2048:2560] = g["c3_gb"]
    gbias[3072:3328] = g["c4_gb"]
    gbias[3584:3586] = g["cl_gb"]
    gbias = gbias.reshape(29, 128)
    gb_zero = not np.any(gbias[0:28])

    # conv weights -> [11, ci, co] tap-major with zero padding
    convt = np.zeros((11, F, F), np.float32)
    convt[5, :, 0:512] = g["conv1_w"][:, :, 0].T
    for t in range(3):
        convt[t + 4, :, 512:768] = g["conv2_w"][:, :, t].T
    for t in range(5):
        convt[t + 3, :, 768:832] = g["conv3_w"][:, :, t].T
    for t in range(7):
        convt[t + 2, :, 832:896] = g["conv4_w"][:, :, t].T
    for t in range(9):
        convt[t + 1, :, 896:960] = g["conv5_w"][:, :, t].T
    for t in range(11):
        convt[t, :, 960:1024] = g["conv6_w"][:, :, t].T
    # device layout: [P, k, blk, i, m]
    convt8 = np.empty((128, 4, NBLK, 2, 128), NP8)
    for (co, d), idx in BLK.items():
        slab = convt[d + 5, :, co * 128:(co + 1) * 128]  # [F, 128]
        convt8[:, :, idx, :, :] = _f8(
            slab.reshape(4, 2, 128, 128).transpose(2, 0, 1, 3))

    # positional encoding pushed through the convs (host, exact)
    peT = pe.T                             # [F, N] float64
    convt64 = convt.astype(np.float64)
    pe_conv = np.zeros((F, N), np.float64)
    for d in range(-5, 6):
        a, b2 = max(0, -d), N - max(0, d)
        pe_conv[:, a:b2] += convt64[d + 5].T @ peT[:, a + d:b2 + d]
    conv_bias = np.concatenate([g["conv1_b"], g["conv2_b"], g["conv3_b"],
                                g["conv4_b"], g["conv5_b"], g["conv6_b"]])
    c3bias = (g["c3_w"] @ (pe_conv + conv_bias[:, None])
              + g["c3_b"][:, None]).astype(np.float32)   # [C, N]

    # c3 weights + bias block in DR pair layout [k, m, P(5), i, mm]
    c3w8 = np.zeros((128, 4, 5, 2, 128), NP8)
    c3w8[:, :, 0:4] = _f8(np.asarray(g["c3_w"], np.float32).reshape(
        4, 128, 4, 2, 128).transpose(4, 0, 2, 3, 1))
    bias_blk = np.zeros((128, 4, 2, 128), np.float32)
    for t_ in range(24):
        bias_blk[t_, :, 0, :] = c3bias.reshape(4, 128, N)[:, :, t_]
    c3w8[:, :, 4] = _f8(bias_blk)

    c4w8 = _f8(np.asarray(g["c4_w"], np.float32).reshape(
        2, 128, 2, 2, 128).transpose(4, 0, 2, 3, 1))
    clwt = _f32(g["cl_w"].T.reshape(2, 128, PD).transpose(1, 0, 2))

    eb = np.zeros((128, NBC, BC, N), np.float32)
    for k in range(128):
        eb[k, k // BC, k % BC, :] = 1.0
    d8 = np.zeros((128, NBC, 2, BC, N), NP8)
    for k in range(24):
        d8[k, :, 0, :, k] = NP8(1.0)
    d8[:, :, 1] = _f8(eb)
    d8 = d8.reshape(128, NBC, 2, FREE)

    host = dict(M3=M3, v3=v3, s3=s3, gb_zero=gb_zero)

    # xt8: c1 weights + ones/bias rows + per-core x data
    c1wT = np.asarray(g["c1_w"], np.float32).T           # [2, 1024]
    c1b = np.asarray(g["c1_b"], np.float32)
    xt_all = x.transpose(2, 0, 1).reshape(PD, B * N)     # [2, B*N]

    shared = dict(wg8=wg8, wgl=wgl, convt8=convt8,
                  c3w8=c3w8, c4w8=c4w8, clwt=clwt, d8=d8)
    in_maps = []
    for k in range(NCORES):
        sl = slice(k * BLOC, (k + 1) * BLOC)
        xt8 = np.zeros((2, 2, F + BLOC * N), NP8)
        xt8[:, 0, 0:F] = _f8(c1wT)
        xt8[0, 1, 0:F] = _f8(c1b)
        xt8[:, 0, F:] = _f8(xt_all[:, k * BLOC * N:(k + 1) * BLOC * N])
        xt8[0, 1, F:] = NP8(1.0)
        smalls = np.zeros((BLOC, 46), np.float32)
        smalls[:, 0] = beta[sl]
        smalls[:, 1:30] = gbias.T
        smalls[:, 30:32] = _f32(g["c4_b"].reshape(2, 128)).T
        smalls[0:PD, 32] = _f32(g["cl_b"])
        for k in range(3):
            smalls[:, 33 + 3 * k:36 + 3 * k] = M3[:, k][None, :]
        smalls[:, 42:45] = v3[None, :]
        smalls[:, 45] = math.pi / 2
        m = dict(shared)
        m["ctx"] = np.ascontiguousarray(context[sl])
        m["smalls"] = smalls
        m["xt8"] = xt8
        in_maps.append(m)

    return host, in_maps


_LAST_HOST = None


def kernel(**inputs):
    global LAST_RESULTS, _LAST_HOST
    host, in_maps = _host_prep(**inputs)
    _LAST_HOST = host
    trace = bool(int(os.environ.get("KERNEL_TRACE", "0")))
    res = _build_and_run(host, in_maps, trace)
    LAST_RESULTS = res
    out = np.concatenate(
        [res.results[k]["out"].reshape(PD, BLOC, N).transpose(1, 2, 0)
         for k in range(NCORES)], axis=0)
    return out


# revision 84
# speedup vs baseline: 1.0003x; 1.0003x over previous
"""Trainium2 Bass kernel for nn_CNNConcatLinear (B=1024, N=24, PD=2, C=512).

Strategy: pure data-parallel over batch (128 per core x 8 cores).

Per core, channels-on-partitions layouts with fp8e4m3 DoubleRow matmuls
(2x128 contraction per instruction at 0.5 cycles/output-row) for every
heavy contraction: the c1 ConcatSquashLinear, the 6 merged convolutions,
c3, c4, and the phase-B gate/hyper-bias matmuls. The final cl layer stays
in f32r (its hyper path dominates the error budget), with bf16 gate
matmuls for cl.

  phase A: new_ctx = context + (sum_j e_j t_j)/(sum_j e_j) -- the two 1x1
           convs + softmax collapse algebraically into a 3x3 system folded
           on host.
  phase B: all gates/hyper-biases as [feature, batch] fp8-DR matmuls from
           new_ctx^T; batched group epilogues (sigmoid on Act, copies on
           DVE) when the gate biases are all zero (they are, per the
           reference init).
  phase C: pipeline over batch chunks of 16:
           c1 (one DR matmul incl. the bias via a ones-row) -> gated X
           [f, b, n] fp8 -> convs as 28 (co, tap) blocks x 4 ci-pair DR
           matmuls -> c3 (4 m-chunks x 4 pair DR + 1 bias-fold DR against
           a delta tile; conv bias + positional encoding folded into the
           c3 bias table on host) -> c4 -> cl (f32r).

Output is written DMA-friendly as [PD, BLOC*N] per core and transposed
back to [BLOC, N, PD] on host.
"""

import math
import os

import numpy as np
import ml_dtypes

B, N, PD, C = 1024, 24, 2, 512
F = 2 * C
NCORES = 8
BLOC = B // NCORES          # 128 batch per core
BC = 16                     # batch chunk
NBC = BLOC // BC            # 8 chunks
PADL = 8
NW = N + 2 * PADL           # 40 (padded X width)
FREE = BC * N               # 384

TAPS = {
    0: [0], 1: [0], 2: [0], 3: [0],
    4: [0, -1, 1], 5: [0, -1, 1],
    6: [0, -1, 1, -2, 2, -3, 3],
    7: [0, -1, 1, -2, 2, -3, 3, -4, 4, -5, 5],
}
BLK = {}
for _co in range(8):
    for _d in TAPS[_co]:
        BLK[(_co, _d)] = len(BLK)
NBLK = len(BLK)             # 28

NP8 = ml_dtypes.float8_e4m3
NPBF = ml_dtypes.bfloat16

LAST_RESULTS = None         # BassKernelResults from the most recent run


def _pe_table():
    pos = np.arange(N, dtype=np.float32)[:, None]
    div = np.exp(np.arange(0, F, 2, dtype=np.float32) * (-np.log(10000.0) / F))
    pe = np.zeros((N, F), dtype=np.float32)
    pe[:, 0::2] = np.sin(pos * div)
    pe[:, 1::2] = np.cos(pos * div)
    return pe


def _f32(a):
    return np.ascontiguousarray(np.asarray(a, dtype=np.float32))


def _f8(a):
    return np.ascontiguousarray(np.asarray(np.asarray(a, np.float32), dtype=NP8))


def _build(host, num_devices=NCORES):
    import concourse.bass as bass
    import concourse.mybir as mybir
    import concourse.tile as tile
    from concourse import bacc
    from concourse.masks import make_identity

    f32 = mybir.dt.float32
    f32r = mybir.dt.float32r
    f8 = mybir.dt.float8e4
    bf16 = mybir.dt.bfloat16
    DR = mybir.MatmulPerfMode.DoubleRow
    AluOp = mybir.AluOpType
    Act = mybir.ActivationFunctionType

    M3, v3, s3 = host["M3"], host["v3"], host["s3"]
    GB_ZERO = host["gb_zero"]

    nc = bacc.Bacc("TRN2", target_bir_lowering=False, debug=False,
                   num_devices=num_devices)

    def din(name, shape, dt):
        return nc.dram_tensor(name, list(shape), dt, kind="ExternalInput").ap()

    ctx_d = din("ctx", [BLOC, C], f32)
    smalls_d = din("smalls", [BLOC, 46], f32)
    xt8_d = din("xt8", [2, 2, F + BLOC * N], f8)
    wg8_d = din("wg8", [128, 28, 2, 2, 128], f8)
    wgl_d = din("wgl", [128, 4, 2, 2], bf16)
    convt8_d = din("convt8", [128, 4, NBLK, 2, 128], f8)
    c3w8_d = din("c3w8", [128, 4, 5, 2, 128], f8)
    c4w8_d = din("c4w8", [128, 2, 2, 2, 128], f8)
    clwt_d = din("clwt", [128, 2, PD], f32r)
    d8_d = din("d8", [128, NBC, 2, FREE], f8)
    out_d = nc.dram_tensor("out", [PD, BLOC * N], f32,
                           kind="ExternalOutput").ap()
    DEBUG = bool(int(os.environ.get("KERNEL_DEBUG", "0")))
    if DEBUG:
        dbg_nctx = nc.dram_tensor("dbg_nctx", [128, C], f32,
                                  kind="ExternalOutput").ap()
        dbg_g = nc.dram_tensor("dbg_g", [128, 2048], f32,
                               kind="ExternalOutput").ap()
        dbg_x = nc.dram_tensor("dbg_x", [4, 128, 2, BC, NW], f32,
                               kind="ExternalOutput").ap()
        dbg_y = nc.dram_tensor("dbg_y", [4, 128, 2, BC, N], f32,
                               kind="ExternalOutput").ap()
        dbg_t3 = nc.dram_tensor("dbg_t3", [2, 128, 2, BC, N], f32,
                                kind="ExternalOutput").ap()
        dbg_t4 = nc.dram_tensor("dbg_t4", [2, 128, BC, N], f32,
                                kind="ExternalOutput").ap()
        dbg_gw = nc.dram_tensor("dbg_gw", [4, 128, 2, 2, 128], f32,
                                kind="ExternalOutput").ap()
        dbg_n8 = nc.dram_tensor("dbg_n8", [128, 4, 128], f32,
                                kind="ExternalOutput").ap()
        dbg_wgd = nc.dram_tensor("dbg_wgd", [128, 4, 2, 2, 128], f8,
                                 kind="ExternalOutput").ap()

    with tile.TileContext(nc) as tc:
        import contextlib
        est = contextlib.ExitStack()
        with est:
            wp = est.enter_context(tc.tile_pool(name="wp", bufs=1))
            gout = est.enter_context(tc.tile_pool(name="gout", bufs=1))

            # ---------- persistent small tiles + their DMAs ----------
            smalls = wp.tile([128, 46], f32, tag="smalls")
            nc.sync.dma_start(smalls[:], smalls_d[:])
            beta_t0 = smalls[:, 0:1]
            gbias_s = smalls[:, 1:30]
            c4b_s = smalls[:, 30:32]
            clb_s = smalls[0:PD, 32:33]
            ctx_t0 = wp.tile([128, C], f32, tag="ctx")
            nc.sync.dma_start(ctx_t0[:], ctx_d[:])
            ident = wp.tile([128, 128], f32, tag="ident")
            make_identity(nc, ident[:])

            # gate/hyper output tiles [feature_part, chunk*128 + b]
            g1b1_s = gout.tile([128, 2048], f32, tag="g1b1")
            g3h3_s = gout.tile([128, 1024], f32, tag="g3h3")
            g4h4_s = gout.tile([128, 512], f32, tag="g4h4")
            gl_s = gout.tile([PD, 128], f32, tag="gl")
            hl_s = gout.tile([PD, 128], f32, tag="hl")

            # ---------- conv weight tiles (fp8 pairs) ----------
            convw_all = wp.tile([128, 4, NBLK, 2, 128], f8, tag="convw")
            convw_s = [convw_all[:, P] for P in range(4)]

            # ---------- phase-C SBUF pools (open early so c1(0) can run in B)
            xp = est.enter_context(tc.tile_pool(name="xp", bufs=1))
            tmpp = est.enter_context(tc.tile_pool(name="tmpp", bufs=6))
            dbgp = est.enter_context(tc.tile_pool(name="dbgp", bufs=1)) \
                if DEBUG else None

            def bcast(ap_2d, np_=N):
                return ap_2d.unsqueeze(2).broadcast_to(
                    [ap_2d.shape[0], BC, np_])


            def _c1_step(bc, fc, pspool, pstag="c1"):
                cs = bc * BC
                X_t = X_gen[bc % 2]
                ps1 = pspool.tile([128, BC, N], f32, tag=pstag)
                nc.tensor.matmul(
                    ps1[:].rearrange("p b n -> p (b n)"), xt8_w[:, :, fc, :],
                    xt8_s[:, :, F + cs * N:F + cs * N + FREE],
                    start=True, stop=True, perf_mode=DR)
                g1 = bcast(g1b1_s[:, fc * 128 + cs:fc * 128 + cs + BC])
                b1 = bcast(g1b1_s[:, 1024 + fc * 128 + cs:1024 + fc * 128 + cs + BC])
                tmp = tmpp.tile([128, BC, N], f32, tag="c1tmp")
                nc.vector.tensor_mul(tmp[:], ps1[:], g1)
                xi = X_t[fc // 2][:, fc % 2, :, PADL:PADL + N]
                (nc.gpsimd if fc % 4 < 2 else nc.vector).tensor_add(
                    xi, tmp[:], b1)

            # ---------- phase A: new_ctx ----------
            with tc.tile_pool(name="pa", bufs=1) as pap:
                ctx_t = ctx_t0
                beta_t = beta_t0

                sinb = pap.tile([128, 1], f32, tag="sinb")
                nc.scalar.activation(sinb[:], beta_t[:], Act.Sin)
                cosb = pap.tile([128, 1], f32, tag="cosb")
                nc.scalar.activation(cosb[:], beta_t[:], Act.Sin,
                                     bias=smalls[:, 45:46])

                u = pap.tile([128, 3], f32, tag="u")
                cm3_0, cm3_1, cm3_2 = (smalls[:, 33 + 3 * k:36 + 3 * k]
                                       for k in range(3))
                v3_t = smalls[:, 42:45]
                nc.vector.scalar_tensor_tensor(u[:], cm3_0, beta_t[:], v3_t,
                                               AluOp.mult, AluOp.add)
                nc.vector.scalar_tensor_tensor(u[:], cm3_1, sinb[:], u[:],
                                               AluOp.mult, AluOp.add)
                nc.vector.scalar_tensor_tensor(u[:], cm3_2, cosb[:], u[:],
                                               AluOp.mult, AluOp.add)

                psa_cm = tc.tile_pool(name="ps_a", bufs=4, space="PSUM")
                psa = psa_cm.__enter__()
                ej = psa.tile([128, C], f32, tag="ej")
                z = pap.tile([128, C], f32, tag="z")
                num = pap.tile([128, C], f32, tag="num")
                tvec = [beta_t, sinb, cosb]
                for j in range(3):
                    nc.scalar.activation(ej[:], ctx_t[:], Act.Exp,
                                         bias=u[:, j:j + 1], scale=float(s3[j]))
                    if j == 0:
                        nc.vector.tensor_copy(z[:], ej[:])
                        nc.vector.tensor_scalar(num[:], ej[:], tvec[j][:], None,
                                                AluOp.mult)
                    else:
                        nc.vector.tensor_add(z[:], z[:], ej[:])
                        nc.vector.scalar_tensor_tensor(num[:], ej[:], tvec[j][:],
                                                       num[:], AluOp.mult,
                                                       AluOp.add)
                nc.vector.reciprocal(z[:], z[:])
                nc.vector.tensor_mul(num[:], num[:], z[:])
                nctx = ctx_t
                nc.vector.tensor_add(nctx[:], ctx_t[:], num[:])

                nctx8 = wp.tile([128, 4, 128], f8, tag="nctx8")
                nctxbf = wp.tile([128, 4, 128], bf16, tag="nctxbf")
                for kb in range(4):
                    pst = psa.tile([128, 128], f32, tag="tr")
                    nc.tensor.transpose(pst[:],
                                        nctx[:, kb * 128:(kb + 1) * 128],
                                        ident[:])
                    nc.vector.tensor_copy(nctx8[:, kb, :], pst[:])
                    nc.scalar.copy(nctxbf[:, kb, :], pst[:])
                if DEBUG:
                    dbg_nx0 = nctx
                psa_cm.__exit__(None, None, None)
            nctx8p = nctx8[:].rearrange("p (P i) b -> p P i b", P=2)

            # ---------- bulk weight/data DMAs: one ordered SP stream
            # (DMA transfers serialize on the shared DMA engines; order by
            # first use). xt8 rides the Pool SWDGE queue in parallel.
            xt8_s = wp.tile([2, 2, F + BLOC * N], f8, tag="xt8")
            nc.gpsimd.dma_start(xt8_s[:], xt8_d[:])
            gw_all = wp.tile([128, 28, 2, 2, 128], f8, tag="gw")
            for q in range(4):
                nc.sync.dma_start(gw_all[:, 7 * q:7 * q + 7],
                                  wg8_d[:, 7 * q:7 * q + 7])
            wgl_s = wp.tile([128, 4, 2, 2], bf16, tag="wgl")
            nc.sync.dma_start(wgl_s[:], wgl_d[:])
            for q in range(4):
                nc.sync.dma_start(convw_all[:, :, 7 * q:7 * q + 7],
                                  convt8_d[:, :, 7 * q:7 * q + 7])
            c3w_all = wp.tile([128, 4, 5, 2, 128], f8, tag="c3w")
            nc.sync.dma_start(c3w_all[:], c3w8_d[:])
            c4w_all = wp.tile([128, 2, 2, 2, 128], f8, tag="c4w")
            nc.sync.dma_start(c4w_all[:], c4w8_d[:])
            d8_s = wp.tile([128, NBC, 2, FREE], f8, tag="d8")
            nc.sync.dma_start(d8_s[:], d8_d[:])
            clw_all = wp.tile([128, 2, PD], f32r, tag="clw")
            nc.sync.dma_start(clw_all[:], clwt_d[:])

            X_gen = []
            for gen in range(2):
                gtiles = []
                for P in range(4):
                    Xf = xp.tile([128, 2, BC, NW], f8, tag=f"x{gen}_{P}")
                    nc.gpsimd.memset(Xf[:, :, :, 0:PADL].bitcast(f32), 0.0)
                    nc.gpsimd.memset(Xf[:, :, :, PADL + N:NW].bitcast(f32), 0.0)
                    gtiles.append(Xf)
                X_gen.append(gtiles)
            xt8_w = xt8_s[:, :, 0:F].rearrange("p i (fc m) -> p i fc m", fc=8)

            # ---------- phase B: gates ----------
            ps_c1 = est.enter_context(
                tc.tile_pool(name="ps_c1", bufs=2, space="PSUM"))
            with tc.tile_pool(name="ps_b", bufs=5, space="PSUM") as pbp:
                # 4 chunk-serial chains per bank tile, one batched drain per
                # group (safe: chains are serial and fully stopped before the
                # group read)
                GDEST = {}
                for c in range(28):
                    if c < 8:
                        GDEST[c] = (g1b1_s, c * 128, True)
                    elif c < 16:
                        GDEST[c] = (g1b1_s, 1024 + (c - 8) * 128, False)
                    elif c < 20:
                        GDEST[c] = (g3h3_s, (c - 16) * 128, True)
                    elif c < 24:
                        GDEST[c] = (g3h3_s, 512 + (c - 20) * 128, False)
                    elif c < 26:
                        GDEST[c] = (g4h4_s, (c - 24) * 128, True)
                    else:
                        GDEST[c] = (g4h4_s, 256 + (c - 26) * 128, False)
                gp_bank = None
                ORDER = [8, 9, 10, 11, 0, 1, 2, 3, 12, 13, 14, 15, 4, 5, 6, 7]
                ORDER += list(range(16, 28))
                C1_B = {3: [0, 1, 2, 3], 7: [4, 5, 6, 7]}
                for idx, c in enumerate(ORDER):
                    if idx % 4 == 0:
                        gp_bank = pbp.tile([128, 4, 128], f32, tag="gps")
                    for P in range(2):
                        nc.tensor.matmul(gp_bank[:, idx % 4, :],
                                         gw_all[:, c, P], nctx8p[:, P],
                                         start=(P == 0), stop=(P == 1),
                                         perf_mode=DR)
                    if idx % 4 == 3 and GB_ZERO and c != 27:
                        dst_t, off, is_g = GDEST[c - 3]
                        dst = dst_t[:, off:off + 512]
                        src = gp_bank[:].rearrange("p c b -> p (c b)")
                        if is_g:
                            nc.scalar.activation(dst, src, Act.Sigmoid)
                        else:
                            nc.vector.tensor_copy(dst, src)
                    elif idx % 4 == 3 and GB_ZERO:
                        nc.scalar.activation(
                            g4h4_s[:, 0:256],
                            gp_bank[:, 0:2].rearrange("p c b -> p (c b)"),
                            Act.Sigmoid)
                        nc.vector.tensor_copy(
                            g4h4_s[:, 256:512],
                            gp_bank[:, 2:4].rearrange("p c b -> p (c b)"))
                    elif idx % 4 == 3:
                        for cc in range(c - 3, c + 1):
                            dst_t, off, is_g = GDEST[cc]
                            dst = dst_t[:, off:off + 128]
                            if is_g:
                                nc.scalar.activation(
                                    dst, gp_bank[:, cc % 4, :], Act.Sigmoid,
                                    bias=gbias_s[:, cc:cc + 1])
                            else:
                                nc.vector.tensor_copy(dst, gp_bank[:, cc % 4, :])
                    for fc in C1_B.get(c if idx % 4 == 3 else -1, []):
                        _c1_step(0, fc, ps_c1)

                # cl gates in bf16: two [2, 128] psums (partition-0 aligned)
                psl_g_full = pbp.tile([128, 128], f32, tag="gps")
                psl_h_full = pbp.tile([128, 128], f32, tag="gps")
                psl_g = psl_g_full[0:2]
                psl_h = psl_h_full[0:2]
                for kb in range(4):
                    nc.tensor.matmul(psl_g[:], wgl_s[:, kb, 0, :],
                                     nctxbf[:, kb, :],
                                     start=(kb == 0), stop=(kb == 3))
                for kb in range(4):
                    nc.tensor.matmul(psl_h[:], wgl_s[:, kb, 1, :],
                                     nctxbf[:, kb, :],
                                     start=(kb == 0), stop=(kb == 3))
                nc.scalar.activation(gl_s[:], psl_g[:], Act.Sigmoid,
                                     bias=gbias_s[0:2, 28:29])
                nc.scalar.copy(hl_s[:], psl_h[:])

                # h/g folds: a*g + h == (a + h/g)*g, so h/g (and c4b) ride the
                # psum via delta-tile matmuls and each CSL epilogue is one mul
                h3g = wp.tile([128, 512], f32, tag="h3g")
                nc.vector.reciprocal(h3g[:], g3h3_s[:, 0:512])
                nc.vector.tensor_mul(h3g[:], h3g[:], g3h3_s[:, 512:1024])
                h4g = wp.tile([128, 256], f32, tag="h4g")
                nc.vector.reciprocal(h4g[:], g4h4_s[:, 0:256])
                nc.vector.tensor_mul(h4g[:], h4g[:], g4h4_s[:, 256:512])
                for mc in range(2):
                    nc.vector.tensor_scalar(
                        h4g[:, mc * 128:(mc + 1) * 128],
                        h4g[:, mc * 128:(mc + 1) * 128],
                        c4b_s[:, mc:mc + 1], None, AluOp.add)
                # write folds into plane 1 of the combined delta-weight
                # tiles (plane 0 carries the c3 bias / zeros)
                c4f = wp.tile([128, 2, 2, 128], f8, tag="c4f")
                nc.vector.memset(c4f[:, :, 0, :].bitcast(f32), 0.0)
                trh_cm = tc.tile_pool(name="ps_t", bufs=1, space="PSUM")
                trhp = trh_cm.__enter__()
                for m in range(4):
                    pst = trhp.tile([128, 128], f32, tag="trh")
                    nc.tensor.transpose(pst[:], h3g[:, m * 128:(m + 1) * 128],
                                        ident[:])
                    nc.vector.tensor_copy(c3w_all[:, m, 4, 1, :], pst[:])
                for mc in range(2):
                    pst = trhp.tile([128, 128], f32, tag="trh")
                    nc.tensor.transpose(pst[:], h4g[:, mc * 128:(mc + 1) * 128],
                                        ident[:])
                    nc.vector.tensor_copy(c4f[:, mc, 1, :], pst[:])
                trh_cm.__exit__(None, None, None)
            if DEBUG:
                nc.sync.dma_start(dbg_nctx[:], dbg_nx0[:])
                nc.sync.dma_start(dbg_g[:], g1b1_s[:])
                for c in range(4):
                    stg_gw = dbgp.tile([128, 2, 2, 128], f32, tag="dgw",
                                       name="stggw")
                    nc.scalar.copy(stg_gw[:], gw_all[:, c])
                    nc.sync.dma_start(dbg_gw[c], stg_gw[:])
                stg_n8 = dbgp.tile([128, 4, 128], f32, tag="dn8", name="stgn8")
                nc.scalar.copy(stg_n8[:], nctx8[:])
                nc.sync.dma_start(dbg_n8[:], stg_n8[:])
                nc.sync.dma_start(dbg_wgd[:], wg8_d[:, 0:4])

            # ---------- phase C ----------
            C1_AT = {1: [0], 2: [1], 3: [2, 3], 4: [4], 5: [5], 6: [6], 7: [7]}
            with (
                tc.tile_pool(name="yp", bufs=2) as yp,
                tc.tile_pool(name="t3p", bufs=2) as t3p,
                tc.tile_pool(name="t4p", bufs=2) as t4p,
                tc.tile_pool(name="ofp", bufs=3) as ofp,
                tc.tile_pool(name="ps_cv", bufs=4, space="PSUM") as ps_cv,
                tc.tile_pool(name="ps_c3", bufs=2, space="PSUM") as ps_c3,
            ):
                def _dbg_dump(dst, tiles, shape):
                    for i, t in enumerate(tiles):
                        stg = dbgp.tile(shape, f32, tag="dbg", name="dbgstg")
                        nc.scalar.copy(stg[:], t[:])
                        nc.sync.dma_start(dst[i], stg[:])

                def emit_c3(bc, Y_t):
                    cs = bc * BC
                    T3_t = [t3p.tile([128, 2, BC, N], f8, tag=f"t3{P}",
                                     name=f"t3{P}") for P in range(2)]
                    for m in range(4):
                        ps3 = ps_c3.tile([128, BC, N], f32, tag="c3")
                        ps3f = ps3[:].rearrange("p b n -> p (b n)")
                        for P in range(4):
                            nc.tensor.matmul(
                                ps3f, c3w_all[:, m, P],
                                Y_t[P][:].rearrange("p i b n -> p i (b n)"),
                                start=(P == 0), stop=False, perf_mode=DR)
                        nc.tensor.matmul(
                            ps3f, c3w_all[:, m, 4], d8_s[:, bc],
                            start=False, stop=True, perf_mode=DR)
                        g3 = bcast(g3h3_s[:, m * 128 + cs:m * 128 + cs + BC])
                        nc.vector.tensor_mul(T3_t[m // 2][:, m % 2], ps3[:], g3)
                    if DEBUG and bc == 0:
                        _dbg_dump(dbg_t3, T3_t, [128, 2, BC, N])
                    return T3_t

                def emit_c4(bc, T3_t):
                    cs = bc * BC
                    T4_t = [t4p.tile([128, BC, N], f32r, tag=f"t4{mc}",
                                     name=f"t4{mc}") for mc in range(2)]
                    for mc in range(2):
                        ps4 = ps_c3.tile([128, BC, N], f32, tag="c3")
                        ps4f = ps4[:].rearrange("p b n -> p (b n)")
                        for P in range(2):
                            nc.tensor.matmul(
                                ps4f, c4w_all[:, mc, P],
                                T3_t[P][:].rearrange("p i b n -> p i (b n)"),
                                start=(P == 0), stop=False, perf_mode=DR)
                        nc.tensor.matmul(
                            ps4f, c4f[:, mc], d8_s[:, bc],
                            start=False, stop=True, perf_mode=DR)
                        g4 = bcast(g4h4_s[:, mc * 128 + cs:mc * 128 + cs + BC])
                        nc.vector.tensor_mul(T4_t[mc][:], ps4[:], g4)
                    if DEBUG and bc == 0:
                        _dbg_dump(dbg_t4, T4_t, [128, BC, N])
                    return T4_t

                def emit_cl(bc, T4_t):
                    cs = bc * BC
                    psl_full = ps_c3.tile([128, BC, N], f32, tag="c3")
                    psl = psl_full[0:PD]
                    for k in range(2):
                        nc.tensor.matmul(psl[:], clw_all[:, k, :], T4_t[k][:],
                                         start=(k == 0), stop=(k == 1))
                    OF = ofp.tile([PD, BC, N], f32, tag="of")
                    gl = gl_s[:, cs:cs + BC].unsqueeze(2).broadcast_to([PD, BC, N])
                    hl = hl_s[:, cs:cs + BC].unsqueeze(2).broadcast_to([PD, BC, N])
                    nc.vector.scalar_tensor_tensor(OF[:], psl[:], clb_s[:], gl,
                                                   AluOp.add, AluOp.mult)
                    (nc.vector if bc >= NBC - 2 else nc.gpsimd).tensor_add(
                        OF[:], OF[:], hl)
                    nc.sync.dma_start(
                        out_d[:, bc * FREE:(bc + 1) * FREE],
                        OF[:].rearrange("p b n -> p (b n)"))

                # 1-chunk skew: chunk bc-1's c3/c4/cl matmuls are emitted
                # between chunk bc's conv chains so the PE never waits on a
                # freshly-written epilogue output
                stage = {}
                for bc in range(NBC):
                    X_t = X_gen[bc % 2]
                    if DEBUG and bc == 0:
                        _dbg_dump(dbg_x, X_t, [128, 2, BC, NW])

                    Y_t = [yp.tile([128, 2, BC, N], f8, tag=f"y{P}",
                                   name=f"y{P}") for P in range(4)]
                    for co in range(8):
                        if bc + 1 < NBC:
                            for fc in C1_AT.get(co, []):
                                _c1_step(bc + 1, fc, ps_c1)
                        if co == 0 and bc >= 2:
                            emit_cl(bc - 2, stage[bc - 2]["T4"])
                        if bc >= 1:
                            prev = stage[bc - 1]
                            if co == 1:
                                prev["T3"] = emit_c3(bc - 1, prev["Y"])
                            elif co == 6:
                                prev["T4"] = emit_c4(bc - 1, prev["T3"])
                        psc = ps_cv.tile([128, BC, N], f32, tag="conv")
                        mms = [(d, P) for d in TAPS[co] for P in range(4)]
                        for i, (d, P) in enumerate(mms):
                            nc.tensor.matmul(
                                psc[:], convw_s[P][:, BLK[(co, d)], :, :],
                                X_t[P][:, :, :, PADL + d:PADL + d + N],
                                start=(i == 0), stop=(i == len(mms) - 1),
                                perf_mode=DR)
                        nc.scalar.copy(Y_t[co // 2][:, co % 2], psc[:])
                    stage[bc] = {"Y": Y_t}
                    if DEBUG and bc == 0:
                        _dbg_dump(dbg_y, Y_t, [128, 2, BC, N])

                emit_cl(NBC - 2, stage[NBC - 2]["T4"])
                last = stage[NBC - 1]
                last["T3"] = emit_c3(NBC - 1, last["Y"])
                last["T4"] = emit_c4(NBC - 1, last["T3"])
                emit_cl(NBC - 1, last["T4"])

    nc.compile()
    return nc


def _build_and_run(host, in_maps, trace):
    from concourse.bass_utils import run_bass_kernel_spmd

    nc = _build(host)
    res = run_bass_kernel_spmd(
        nc, in_maps, core_ids=list(range(NCORES)), trace=trace,
        trace_cores=list(range(NCORES)) if trace else None,
        stitch_traces=bool(trace and NCORES > 1))
    return res


def _host_prep(**inputs):
    x = _f32(inputs["x"])
    beta = _f32(inputs["beta"])
    context = _f32(inputs["context"])
    g = {k: np.asarray(v, dtype=np.float64) for k, v in inputs.items()
         if k not in ("x", "beta", "context")}

    # --- algebraic folds (host, tiny) ---
    embW = g["emb_w"][:, :, 0]            # [64, 3]
    dembW = g["demb_w"][:, :, 0]          # [3, 64]
    M3 = dembW @ embW                     # [3, 3]
    v3 = dembW @ g["emb_b"] + g["demb_b"]
    s3 = M3.sum(axis=1)

    pe = _pe_table().astype(np.float64)   # [N, F]

    # gate weight matrix WG [C, 28*128] then DR pair layout
    WG = np.concatenate([
        g["c1_gw"].T, g["c1_hw"].T, g["c3_gw"].T, g["c3_hw"].T,
        g["c4_gw"].T, g["c4_hw"].T], axis=1).astype(np.float32)  # [512, 3584]
    wg8 = _f8(WG.reshape(2, 2, 128, 28, 128).transpose(2, 3, 0, 1, 4))
    WGL = np.stack([g["cl_gw"].T, g["cl_hw"].T], axis=1)  # [512, 2, 2]
    wgl = np.ascontiguousarray(
        WGL.reshape(4, 128, 2, 2).transpose(1, 0, 2, 3).astype(NPBF))

    gbias = np.zeros(29 * 128, np.float32)
    gbias[0:1024] = g["c1_gb"]
    gbias[2048:2560] = g["c3_gb"]
    gbias[3072:3328] = g["c4_gb"]
    gbias[3584:3586] = g["cl_gb"]
    gbias = gbias.reshape(29, 128)
    gb_zero = not np.any(gbias[0:28])

    # conv weights -> [11, ci, co] tap-major with zero padding
    convt = np.zeros((11, F, F), np.float32)
    convt[5, :, 0:512] = g["conv1_w"][:, :, 0].T
    for t in range(3):
        convt[t + 4, :, 512:768] = g["conv2_w"][:, :, t].T
    for t in range(5):
        convt[t + 3, :, 768:832] = g["conv3_w"][:, :, t].T
    for t in range(7):
        convt[t + 2, :, 832:896] = g["conv4_w"][:, :, t].T
    for t in range(9):
        convt[t + 1, :, 896:960] = g["conv5_w"][:, :, t].T
    for t in range(11):
        convt[t, :, 960:1024] = g["conv6_w"][:, :, t].T
    # device layout: [P, k, blk, i, m]
    convt8 = np.empty((128, 4, NBLK, 2, 128), NP8)
    for (co, d), idx in BLK.items():
        slab = convt[d + 5, :, co * 128:(co + 1) * 128]  # [F, 128]
        convt8[:, :, idx, :, :] = _f8(
            slab.reshape(4, 2, 128, 128).transpose(2, 0, 1, 3))

    # positional encoding pushed through the convs (host, exact)
    peT = pe.T                             # [F, N] float64
    convt64 = convt.astype(np.float64)
    pe_conv = np.zeros((F, N), np.float64)
    for d in range(-5, 6):
        a, b2 = max(0, -d), N - max(0, d)
        pe_conv[:, a:b2] += convt64[d + 5].T @ peT[:, a + d:b2 + d]
    conv_bias = np.concatenate([g["conv1_b"], g["conv2_b"], g["conv3_b"],
                                g["conv4_b"], g["conv5_b"], g["conv6_b"]])
    c3bias = (g["c3_w"] @ (pe_conv + conv_bias[:, None])
              + g["c3_b"][:, None]).astype(np.float32)   # [C, N]

    # c3 weights + bias block in DR pair layout [k, m, P(5), i, mm]
    c3w8 = np.zeros((128, 4, 5, 2, 128), NP8)
    c3w8[:, :, 0:4] = _f8(np.asarray(g["c3_w"], np.float32).reshape(
        4, 128, 4, 2, 128).transpose(4, 0, 2, 3, 1))
    bias_blk = np.zeros((128, 4, 2, 128), np.float32)
    for t_ in range(24):
        bias_blk[t_, :, 0, :] = c3bias.reshape(4, 128, N)[:, :, t_]
    c3w8[:, :, 4] = _f8(bias_blk)

    c4w8 = _f8(np.asarray(g["c4_w"], np.float32).reshape(
        2, 128, 2, 2, 128).transpose(4, 0, 2, 3, 1))
    clwt = _f32(g["cl_w"].T.reshape(2, 128, PD).transpose(1, 0, 2))

    eb = np.zeros((128, NBC, BC, N), np.float32)
    for k in range(128):
        eb[k, k // BC, k % BC, :] = 1.0
    d8 = np.zeros((128, NBC, 2, BC, N), NP8)
    for k in range(24):
        d8[k, :, 0, :, k] = NP8(1.0)
    d8[:, :, 1] = _f8(eb)
    d8 = d8.reshape(128, NBC, 2, FREE)

    host = dict(M3=M3, v3=v3, s3=s3, gb_zero=gb_zero)

    # xt8: c1 weights + ones/bias rows + per-core x data
    c1wT = np.asarray(g["c1_w"], np.float32).T           # [2, 1024]
    c1b = np.asarray(g["c1_b"], np.float32)
    xt_all = x.transpose(2, 0, 1).reshape(PD, B * N)     # [2, B*N]

    shared = dict(wg8=wg8, wgl=wgl, convt8=convt8,
                  c3w8=c3w8, c4w8=c4w8, clwt=clwt, d8=d8)
    in_maps = []
    for k in range(NCORES):
        sl = slice(k * BLOC, (k + 1) * BLOC)
        xt8 = np.zeros((2, 2, F + BLOC * N), NP8)
        xt8[:, 0, 0:F] = _f8(c1wT)
        xt8[0, 1, 0:F] = _f8(c1b)
        xt8[:, 0, F:] = _f8(xt_all[:, k * BLOC * N:(k + 1) * BLOC * N])
        xt8[0, 1, F:] = NP8(1.0)
        smalls = np.zeros((BLOC, 46), np.float32)
        smalls[:, 0] = beta[sl]
        smalls[:, 1:30] = gbias.T
        smalls[:, 30:32] = _f32(g["c4_b"].reshape(2, 128)).T
        smalls[0:PD, 32] = _f32(g["cl_b"])
        for k in range(3):
            smalls[:, 33 + 3 * k:36 + 3 * k] = M3[:, k][None, :]
        smalls[:, 42:45] = v3[None, :]
        smalls[:, 45] = math.pi / 2
        m = dict(shared)
        m["ctx"] = np.ascontiguousarray(context[sl])
        m["smalls"] = smalls
        m["xt8"] = xt8
        in_maps.append(m)

    return host, in_maps


_LAST_HOST = None


def kernel(**inputs):
    global LAST_RESULTS, _LAST_HOST
    host, in_maps = _host_prep(**inputs)
    _LAST_HOST = host
    trace = bool(int(os.environ.get("KERNEL_TRACE", "0")))
    res = _build_and_run(host, in_maps, trace)
    LAST_RESULTS = res
    out = np.concatenate(
        [res.results[k]["out"].reshape(PD, BLOC, N).transpose(1, 2, 0)
         for k in range(NCORES)], axis=0)
    return out
